# revision 1
# baseline (speedup 1.0000x reference)
"""Blockwise K/V selector (sparse attention) on 8 Trainium2 NeuronCores.

Full computation on device:
  scores = q . compressed_keys / sqrt(D)  -> softmax -> GQA mean-pool over
  heads -> top-16 blocks (rank trick, no sort) -> indirect-DMA gather of the
  selected 64-row K/V blocks.

Sharding: the 16 (b, g) pairs are fully independent; each of the 8 cores
processes 2 pairs (pure data parallel, no collectives).

Engine placement: loads on the SP HWDGE ring, K stores on SP / V stores on
ACT ring, gathers on the gpsimd SWDGE ring (32-row / 16 KiB descriptors),
scores via PE transposes + per-head matmuls, softmax on ACT, top-k rank
trick on DVE + PE.
"""
import os
import numpy as np

B = 4
H = 32
G = 4
HPG = H // G          # 8 heads per query group
PAIRS = 2             # (b, g) pairs per core
N = 128               # number of compressed keys / key blocks
D = 128               # head dim
S = 8192              # kv sequence length
BS = 64               # block size
NSEL = 16             # selected blocks
NCORES = 8
# gather granularity: 8 rows = 4 KiB per index. The indirect-DMA DGE maps
# one index to one dest SBUF partition, so the per-index span must equal one
# partition line of the dest tile (4 KiB) — larger spans corrupt on HW.
CHUNK = 8
NCHUNK = NSEL * BS // CHUNK   # 128 chunks per pair
RPB = BS // CHUNK     # chunks per block (8)
SCALE = 1.0 / float(D) ** 0.5
GH = PAIRS * HPG      # 16 heads handled per core

# packed constants layout (c_all [128, 387]):
#   0:128 tri | 128:256 noti (1 - I) | 256:384 iotabh (c//RPB)
#   384 pvecr (RPB*p) | 385:387 cvec
CW = 387

_CACHE = {}
LAST_RESULT = None    # BassKernelResults of the most recent run (for test.py)


def _build_nc():
    import concourse.bass as bass
    import concourse.bacc as bacc
    import concourse.mybir as mybir
    import concourse.tile as tile

    F32 = mybir.dt.float32

    nc = bacc.Bacc("TRN2", target_bir_lowering=False, debug=False)

    q_in = nc.dram_tensor("q_in", [PAIRS, HPG, D], F32, kind="ExternalInput")
    ck_in = nc.dram_tensor("ck_in", [PAIRS, HPG, N, D], F32, kind="ExternalInput")
    k_in = nc.dram_tensor("k_in", [PAIRS, S, D], F32, kind="ExternalInput")
    v_in = nc.dram_tensor("v_in", [PAIRS, S, D], F32, kind="ExternalInput")
    c_all = nc.dram_tensor("c_all", [128, CW], F32, kind="ExternalInput")
    out_k = nc.dram_tensor("out_k", [PAIRS, NSEL * BS, D], F32, kind="ExternalOutput")
    out_v = nc.dram_tensor("out_v", [PAIRS, NSEL * BS, D], F32, kind="ExternalOutput")
    dbg = dbg_i = None
    if int(os.environ.get("KDEBUG", "0")):
        dbg = nc.dram_tensor("dbg", [PAIRS, 128, 16], F32, kind="ExternalOutput")
        dbg_i = nc.dram_tensor("dbg_i", [PAIRS, 128, 1], mybir.dt.int32,
                               kind="ExternalOutput")

    # flat chunk views for the gathers: [2*256 chunks, 4096 elems]
    k_flat = k_in[:].rearrange("b (c r) d -> (b c) (r d)", r=CHUNK)
    v_flat = v_in[:].rearrange("b (c r) d -> (b c) (r d)", r=CHUNK)

    # KREPEAT>1 builds the pipeline several times (serialized by the
    # TileContext exit barrier) so device time can be measured as the
    # marginal wall-clock per repeat. KEMPTY=1 emits no-op contexts for
    # calibrating the barrier cost.
    repeat = int(os.environ.get("KREPEAT", "1"))
    empty = bool(int(os.environ.get("KEMPTY", "0")))
    for _rep in range(repeat):
        _emit_once(nc, tc_mod=tile, bassmod=bass, mybirmod=mybir, empty=empty,
                   tensors=(q_in, ck_in, k_flat, v_flat, c_all,
                            out_k, out_v, dbg, dbg_i))

    nc.compile()
    return nc


def _emit_once(nc, tc_mod, bassmod, mybirmod, empty, tensors):
    bass = bassmod
    mybir = mybirmod
    tile = tc_mod
    (q_in, ck_in, k_flat, v_flat, c_all, out_k, out_v, dbg, dbg_i) = tensors
    from concourse.masks import make_identity
    F32 = mybir.dt.float32
    I32 = mybir.dt.int32
    Alu = mybir.AluOpType
    Act = mybir.ActivationFunctionType
    Ax = mybir.AxisListType

    with tile.TileContext(nc) as tc:
        if empty:
            with tc.tile_pool(name="noop", bufs=1) as np_:
                t = np_.tile([1, 1], F32)
                nc.vector.memset(t[:], 0.0)
            return
        with tc.tile_pool(name="consts", bufs=1) as cp, \
             tc.tile_pool(name="work", bufs=2) as wp, \
             tc.tile_pool(name="psckt", bufs=2, space="PSUM") as pck, \
             tc.tile_pool(name="psmid", bufs=2, space="PSUM") as pmid, \
             tc.tile_pool(name="pssm", bufs=2, space="PSUM") as psm:

            # ---- loads (SP ring): q, ident, ck halves, remaining consts ----
            q_sb = wp.tile([GH, D], F32)
            nc.sync.dma_start(out=q_sb[:], in_=q_in[:].rearrange("b h d -> (b h) d"))
            ident = cp.tile([128, 128], F32)
            make_identity(nc, ident[:])
            ck_sb = wp.tile([128, GH * D], F32)
            for p in range(PAIRS):
                nc.sync.dma_start(
                    out=ck_sb[:, p * HPG * D:(p + 1) * HPG * D].rearrange(
                        "n (h d) -> n h d", h=HPG),
                    in_=ck_in[p].rearrange("h n d -> n h d"))
            call = cp.tile([128, CW], F32)
            nc.sync.dma_start(out=call[:], in_=c_all[:])
            tri = call[:, 0:128]
            noti = call[:, 128:256]
            iotabh = call[:, 256:256 + NCHUNK]
            pvecr = call[:, 384:385]
            cvec = call[:, 385:387]

            # ---- q^T via PE ----
            qt_ps = psm.tile([D, GH], F32, tag="small")
            nc.tensor.transpose(out=qt_ps[:], in_=q_sb[:], identity=ident[0:GH, 0:GH])
            qt_sb = wp.tile([D, GH], F32)
            nc.vector.tensor_copy(out=qt_sb[:], in_=qt_ps[:])

            for p in range(PAIRS):
                # ---- scoresT[n, h]: transpose ck, one [128,1] matmul/head ----
                ckt_ps = pck.tile([D, HPG * N], F32, tag="ckt")
                for h in range(HPG):
                    nc.tensor.transpose(
                        out=ckt_ps[:, h * N:(h + 1) * N],
                        in_=ck_sb[:, (p * HPG + h) * D:(p * HPG + h + 1) * D],
                        identity=ident[:])
                ckt_sb = wp.tile([D, HPG * N], F32)
                if p == 0:
                    nc.scalar.copy(out=ckt_sb[:], in_=ckt_ps[:])
                else:
                    nc.vector.tensor_copy(out=ckt_sb[:], in_=ckt_ps[:])
                scoresT_ps = pmid.tile([N, HPG], F32, tag="mid")
                for h in range(HPG):
                    nc.tensor.matmul(
                        out=scoresT_ps[:, h:h + 1],
                        lhsT=ckt_sb[:, h * N:(h + 1) * N],
                        rhs=qt_sb[:, p * HPG + h:p * HPG + h + 1],
                        start=True, stop=True)

                # ---- softmax over n without max-subtraction (scores ~ N(0,1)
                # after scaling, exp is overflow-safe; order matches jax to
                # ~1e-7 relative which is far below top-k prob gaps) ----
                ecolT = wp.tile([N, HPG], F32)
                nc.scalar.activation(out=ecolT[:], in_=scoresT_ps[:],
                                     func=Act.Exp, scale=SCALE)
                e_ps = psm.tile([HPG, N], F32, tag="small")
                nc.tensor.transpose(out=e_ps[:], in_=ecolT[:],
                                    identity=ident[:])
                e_sb = wp.tile([HPG, N], F32)
                z = wp.tile([HPG, 1], F32)
                nc.vector.tensor_reduce(out=z[:, :1], in_=e_ps[:],
                                        op=Alu.add, axis=Ax.X)
                nc.vector.tensor_copy(out=e_sb[:], in_=e_ps[:])
                rz = wp.tile([HPG, 1], F32)
                nc.vector.reciprocal(out=rz[:, :1], in_=z[:, :1])

                # ---- pooled (x8, order-preserving) directly in both shapes:
                # A[c] = sum_h e[h,c]*rz[h] (column) and B[r,c] = A[c] (rows)
                # via two matmuls with identical contraction order ----
                b_ps = pmid.tile([128, 128], F32, tag="mid")
                nc.tensor.matmul(out=b_ps[:],
                                 lhsT=rz[:, :1].to_broadcast([HPG, N]),
                                 rhs=e_sb[:], start=True, stop=True)
                a_ps = psm.tile([128, 1], F32, tag="small")
                nc.tensor.matmul(out=a_ps[:], lhsT=e_sb[:], rhs=rz[:, :1],
                                 start=True, stop=True)
                a_sb = wp.tile([128, 1], F32)
                nc.vector.tensor_copy(out=a_sb[:], in_=a_ps[:])
                # A and B are computed by different matmuls whose fp32
                # rounding can differ in the last ulp on HW, so the diagonal
                # self-compare is excluded from the greater-count via (1-I).
                gjunk = wp.tile([128, 128], F32)
                nc.vector.tensor_scalar(
                    out=gjunk[:], in0=b_ps[:], scalar1=a_sb[:, :1], scalar2=None,
                    op0=Alu.is_gt)
                ejunk = wp.tile([128, 128], F32)
                nc.vector.tensor_scalar(
                    out=ejunk[:], in0=b_ps[:], scalar1=a_sb[:, :1], scalar2=None,
                    op0=Alu.is_equal)
                gm = wp.tile([128, 128], F32)
                nc.vector.tensor_tensor(
                    out=gm[:], in0=gjunk[:], in1=noti[:], op=Alu.mult)
                etri = wp.tile([128, 128], F32)
                nc.vector.tensor_tensor(
                    out=etri[:], in0=ejunk[:], in1=tri[:], op=Alu.mult)
                gt = wp.tile([128, 128], F32)
                nc.vector.tensor_tensor(
                    out=gt[:], in0=gm[:], in1=etri[:], op=Alu.add)
                rank = wp.tile([128, 1], F32)
                nc.vector.tensor_reduce(
                    out=rank[:, :1], in_=gt[:], op=Alu.add, axis=Ax.X)

                # ---- selection matrix -> chunk bases in one matmul:
                # chunk[c] = sum_p [rank[p] == c//RPB] * (RPB*p) ----
                sel = wp.tile([128, NCHUNK], F32)
                nc.vector.tensor_scalar(
                    out=sel[:], in0=iotabh[:], scalar1=rank[:, :1], scalar2=None,
                    op0=Alu.is_equal)
                chunk_ps = psm.tile([NCHUNK, 1], F32, tag="small")
                nc.tensor.matmul(out=chunk_ps[:], lhsT=sel[:], rhs=pvecr[:],
                                 start=True, stop=True)
                idxi = wp.tile([NCHUNK, 1], I32)
                nc.vector.tensor_tensor(
                    out=idxi[:], in0=chunk_ps[:], in1=cvec[0:NCHUNK, p:p + 1],
                    op=Alu.add)
                if dbg is not None:
                    dwork = wp.tile([128, 16], F32)
                    nc.vector.tensor_copy(out=dwork[:, 0:8], in_=ecolT[:, 0:8])
                    nc.vector.tensor_copy(out=dwork[:, 8:9], in_=a_sb[:, :1])
                    nc.vector.tensor_copy(out=dwork[:, 9:10], in_=rank[:, :1])

                    nc.vector.tensor_copy(out=dwork[:, 12:13], in_=chunk_ps[:])
                    nc.sync.dma_start(out=dbg[p], in_=dwork[:])
                    nc.sync.dma_start(out=dbg_i[p], in_=idxi[:])

                # ---- gather selected blocks (32 chunks x 16 KiB each) ----
                ksel = wp.tile([128, NSEL * BS * D // 128], F32)
                nc.gpsimd.indirect_dma_start(
                    out=ksel[:], out_offset=None, in_=k_flat,
                    in_offset=bass.IndirectOffsetOnAxis(ap=idxi[:, :1], axis=0))
                vsel = wp.tile([128, NSEL * BS * D // 128], F32)
                nc.gpsimd.indirect_dma_start(
                    out=vsel[:], out_offset=None, in_=v_flat,
                    in_offset=bass.IndirectOffsetOnAxis(ap=idxi[:, :1], axis=0))

                # ---- stores: K on SP ring, V on ACT ring ----
                nc.sync.dma_start(
                    out=out_k[p].rearrange("(c r) d -> c (r d)", r=CHUNK // 4),
                    in_=ksel[:])
                nc.scalar.dma_start(
                    out=out_v[p].rearrange("(c r) d -> c (r d)", r=CHUNK // 4),
                    in_=vsel[:])


def _consts():
    call = np.zeros((128, CW), dtype=np.float32)
    call[:, 0:128] = np.tril(np.ones((128, 128), dtype=np.float32), -1)
    call[:, 128:256] = 1.0 - np.eye(128, dtype=np.float32)
    call[:, 256:256 + NCHUNK] = (np.arange(NCHUNK, dtype=np.float32) // RPB)[None, :]
    call[:, 384] = float(RPB) * np.arange(128, dtype=np.float32)
    # cvec[c, p] = p * (S // CHUNK) + c % RPB
    call[:, 385:387] = (np.arange(PAIRS, dtype=np.float32)[None, :] * (S // CHUNK)
                        + (np.arange(128, dtype=np.float32) % RPB)[:, None])
    return {"c_all": call}


def kernel(query, compressed_keys, keys, values):
    global LAST_RESULT
    from concourse.bass_utils import run_bass_kernel_spmd

    query = np.asarray(query, dtype=np.float32)
    compressed_keys = np.asarray(compressed_keys, dtype=np.float32)
    keys = np.asarray(keys, dtype=np.float32)
    values = np.asarray(values, dtype=np.float32)

    key = (os.environ.get("KREPEAT", "1"), os.environ.get("KEMPTY", "0"))
    if key not in _CACHE:
        _CACHE[key] = _build_nc()
    nc = _CACHE[key]

    consts = _consts()
    in_maps = []
    for core in range(NCORES):
        bs, gs = [], []
        for j in range(PAIRS):
            f = PAIRS * core + j
            bs.append(f // G)
            gs.append(f % G)
        q_s = np.stack([query[b, g * HPG:(g + 1) * HPG, -1, :]
                        for b, g in zip(bs, gs)])
        ck_s = np.stack([compressed_keys[b, g * HPG:(g + 1) * HPG]
                         for b, g in zip(bs, gs)])
        k_s = np.stack([keys[b, g] for b, g in zip(bs, gs)])
        v_s = np.stack([values[b, g] for b, g in zip(bs, gs)])
        im = {"q_in": np.ascontiguousarray(q_s),
              "ck_in": np.ascontiguousarray(ck_s),
              "k_in": np.ascontiguousarray(k_s),
              "v_in": np.ascontiguousarray(v_s)}
        im.update(consts)
        in_maps.append(im)

    res = run_bass_kernel_spmd(nc, in_maps, list(range(NCORES)))
    LAST_RESULT = res

    sel_k = np.empty((B, G, NSEL * BS, D), dtype=np.float32)
    sel_v = np.empty((B, G, NSEL * BS, D), dtype=np.float32)
    for core in range(NCORES):
        for j in range(PAIRS):
            f = PAIRS * core + j
            b, g = f // G, f % G
            sel_k[b, g] = res.results[core]["out_k"][j]
            sel_v[b, g] = res.results[core]["out_v"][j]
    return sel_k, sel_v



# revision 26
# speedup vs baseline: 1.1903x; 1.1903x over previous
"""Blockwise K/V selector (sparse attention) on 8 Trainium2 NeuronCores.

Full computation on device:
  scores = q . compressed_keys / sqrt(D)  -> softmax -> GQA mean-pool over
  heads -> top-16 blocks (rank trick, no sort) -> indirect-DMA gather of the
  selected 64-row K/V blocks.

Sharding: the 16 (b, g) pairs are fully independent; each of the 8 cores
processes 2 pairs (pure data parallel, no collectives).

Pipeline (latency-optimized; the gather/store tail overlaps the next
iteration's load/compute phase, so the critical path is loads + the
score->index chain):
  - ck loads split into 4 chunks alternating across the two HWDGE rings
    (SP + ACT) so per-chunk transposes overlap the remaining loads.
  - both (b,g) pairs share one merged instruction chain (scores [n,16],
    one exp, one e-transpose, fused rank reduction via tensor_tensor_reduce).
  - all mask/iota constants generated on-chip (no constant DMA).
  - gathers on SWDGE queue 0, K/V stores on SWDGE queue 1 (issued after all
    gathers in the gpsimd stream to avoid head-of-line blocking), keeping the
    HWDGE rings free for the next iteration's loads.
"""
import os
import numpy as np

B = 4
H = 32
G = 4
HPG = H // G          # 8 heads per query group
PAIRS = 2             # (b, g) pairs per core
N = 128               # number of compressed keys / key blocks
D = 128               # head dim
S = 8192              # kv sequence length
BS = 64               # block size
NSEL = 16             # selected blocks
NCORES = 8
# gather granularity: 8 rows = 4 KiB per index. The indirect-DMA DGE maps
# one index to one dest SBUF partition, so the per-index span must equal one
# partition line of the dest tile (4 KiB) — larger spans corrupt on HW.
CHUNK = 8
NCHUNK = NSEL * BS // CHUNK   # 128 chunks per pair
RPB = BS // CHUNK     # chunks per block (8)
SCALE = 1.0 / float(D) ** 0.5
GH = PAIRS * HPG      # 16 heads handled per core

_CACHE = {}
LAST_RESULT = None    # BassKernelResults of the most recent run (for test.py)


def _build_nc():
    import concourse.bass as bass
    import concourse.bacc as bacc
    import concourse.mybir as mybir
    import concourse.tile as tile

    F32 = mybir.dt.float32

    nc = bacc.Bacc("TRN2", target_bir_lowering=False, debug=False)

    q_in = nc.dram_tensor("q_in", [PAIRS, HPG, D], F32, kind="ExternalInput")
    ck_in = nc.dram_tensor("ck_in", [PAIRS, HPG, N, D], F32, kind="ExternalInput")
    k_in = nc.dram_tensor("k_in", [PAIRS, S, D], F32, kind="ExternalInput")
    v_in = nc.dram_tensor("v_in", [PAIRS, S, D], F32, kind="ExternalInput")
    out_k = nc.dram_tensor("out_k", [PAIRS, NSEL * BS, D], F32, kind="ExternalOutput")
    out_v = nc.dram_tensor("out_v", [PAIRS, NSEL * BS, D], F32, kind="ExternalOutput")
    dbg_i = None
    if int(os.environ.get("KDEBUG", "0")):
        dbg_i = nc.dram_tensor("dbg_i", [NCHUNK, PAIRS], mybir.dt.int32,
                               kind="ExternalOutput")

    # flat chunk views for the gathers: [2*1024 chunks, 1024 elems]
    k_flat = k_in[:].rearrange("b (c r) d -> (b c) (r d)", r=CHUNK)
    v_flat = v_in[:].rearrange("b (c r) d -> (b c) (r d)", r=CHUNK)

    # KREPEAT>1 builds the pipeline several times (serialized by the
    # TileContext exit barrier) so device time can be measured as the
    # marginal wall-clock per repeat. KEMPTY=1 emits no-op contexts for
    # calibrating the barrier cost.
    repeat = int(os.environ.get("KREPEAT", "1"))
    empty = bool(int(os.environ.get("KEMPTY", "0")))
    # KSTAGE: 0=full, 1=loads only, 2=+compute/idx, 3=+gathers (no stores)
    for _rep in range(repeat):
        _emit_once(nc, tc_mod=tile, bassmod=bass, mybirmod=mybir, empty=empty,
                   tensors=(q_in, ck_in, k_flat, v_flat, out_k, out_v, dbg_i))

    nc.compile()
    return nc


def _emit_once(nc, tc_mod, bassmod, mybirmod, empty, tensors):
    bass = bassmod
    mybir = mybirmod
    tile = tc_mod
    (q_in, ck_in, k_flat, v_flat, out_k, out_v, dbg_i) = tensors
    stage = int(os.environ.get("KSTAGE", "0"))
    from concourse.masks import make_identity
    F32 = mybir.dt.float32
    I32 = mybir.dt.int32
    Alu = mybir.AluOpType
    Act = mybir.ActivationFunctionType
    Ax = mybir.AxisListType

    with tile.TileContext(nc) as tc:
        if empty:
            with tc.tile_pool(name="noop", bufs=1) as np_:
                t = np_.tile([1, 1], F32)
                nc.vector.memset(t[:], 0.0)
            return
        with tc.tile_pool(name="consts", bufs=1) as cp, \
             tc.tile_pool(name="work", bufs=2) as wp, \
             tc.tile_pool(name="pckt", bufs=1, space="PSUM") as pck, \
             tc.tile_pool(name="pmid", bufs=2, space="PSUM") as pm, \
             tc.tile_pool(name="psml", bufs=2, space="PSUM") as ps:

            # ---- loads: q leads the SP ring; ck in 4 chunks alternating
            # across the SP / ACT HWDGE rings ----
            q_sb = wp.tile([GH, D], F32)
            nc.sync.dma_start(out=q_sb[:], in_=q_in[:].rearrange("b h d -> (b h) d"))
            ck_sb = []
            for c in range(4):
                p, hh = c // 2, (c % 2) * 4
                t = wp.tile([128, 4 * D], F32, name=f"ck{c}")
                eng = nc.sync if c % 2 == 0 else nc.scalar
                eng.dma_start(
                    out=t[:].rearrange("n (h d) -> n h d", h=4),
                    in_=ck_in[p, hh:hh + 4].rearrange("h n d -> n h d"))
                ck_sb.append(t)

            # ---- constants generated on-chip (cheap; overlaps the loads) ----
            ident = cp.tile([128, 128], F32)
            make_identity(nc, ident[:])
            # tri2[r, (p c)] = 1 iff c < r  (strict lower triangle, both pairs)
            tri2 = cp.tile([128, PAIRS * 128], F32)
            nc.gpsimd.memset(tri2[:], 1.0)
            nc.gpsimd.affine_select(out=tri2[:], in_=tri2[:],
                                    compare_op=Alu.is_gt, fill=0.0, base=0,
                                    pattern=[[0, PAIRS], [-1, 128]],
                                    channel_multiplier=1)
            # noti2[r, (p c)] = 1 iff c != r (diagonal exclusion, both pairs)
            noti2 = cp.tile([128, PAIRS * 128], F32)
            nc.gpsimd.memset(noti2[:], 1.0)
            nc.gpsimd.affine_select(out=noti2[:], in_=noti2[:],
                                    compare_op=Alu.not_equal,
                                    fill=0.0, base=0,
                                    pattern=[[0, PAIRS], [-1, 128]],
                                    channel_multiplier=1)
            # iotabh[r, c] = c // RPB
            iotabh = cp.tile([128, NCHUNK], F32)
            nc.gpsimd.iota(iotabh[:], pattern=[[1, NSEL], [0, RPB]], base=0,
                           channel_multiplier=0,
                           allow_small_or_imprecise_dtypes=True)
            # pvecr[r] = RPB * r
            pvecr = cp.tile([128, 1], F32)
            nc.gpsimd.iota(pvecr[:], pattern=[[0, 1]], base=0,
                           channel_multiplier=RPB,
                           allow_small_or_imprecise_dtypes=True)
            onescol = cp.tile([128, 1], F32)
            nc.gpsimd.memset(onescol[:], 1.0)
            # cvec2[c, p] = (c % RPB) + p * (S // CHUNK)
            it2 = cp.tile([128, PAIRS], I32)
            nc.gpsimd.iota(it2[:], pattern=[[0, PAIRS]], base=0,
                           channel_multiplier=1)
            m8 = cp.tile([128, PAIRS], I32)
            nc.vector.tensor_scalar(out=m8[:], in0=it2[:], scalar1=RPB - 1,
                                    scalar2=None, op0=Alu.bitwise_and)
            pb = cp.tile([128, PAIRS], I32)
            nc.gpsimd.iota(pb[:], pattern=[[S // CHUNK, PAIRS]], base=0,
                           channel_multiplier=0)
            cvec2 = cp.tile([128, PAIRS], F32)
            nc.vector.tensor_tensor(out=cvec2[:], in0=m8[:], in1=pb[:],
                                    op=Alu.add)
            # pmask[h, p] = 1 iff head h belongs to pair p (PE operands must
            # start at partition 0, so pooled matmuls contract over all 16
            # heads with the other pair's terms masked to exact zeros)
            # pmask[h, p] = 1 iff 0 <= h - 8p <= 7, via two affine selects
            pmask = cp.tile([GH, PAIRS], F32)
            nc.gpsimd.memset(pmask[:], 1.0)
            nc.gpsimd.affine_select(out=pmask[:], in_=pmask[:],
                                    compare_op=Alu.is_ge, fill=0.0, base=0,
                                    pattern=[[-HPG, PAIRS]],
                                    channel_multiplier=1)
            nc.gpsimd.affine_select(out=pmask[:], in_=pmask[:],
                                    compare_op=Alu.is_ge, fill=0.0,
                                    base=HPG - 1,
                                    pattern=[[HPG, PAIRS]],
                                    channel_multiplier=-1)

            if stage == 1:
                sink = wp.tile([128, 1], F32)
                nc.vector.tensor_copy(out=sink[:], in_=ck_sb[3][:, 0:1])
                nc.sync.dma_start(out=out_k[0][0:1, 0:1], in_=sink[0:1, 0:1])
                return

            # ---- q^T via PE ----
            qt_ps = ps.tile([D, GH], F32, tag="s")
            nc.tensor.transpose(out=qt_ps[:], in_=q_sb[:],
                                identity=ident[0:GH, 0:GH])
            qt_sb = wp.tile([D, GH], F32)
            nc.vector.tensor_copy(out=qt_sb[:], in_=qt_ps[:])
            if stage == 21:
                nc.sync.dma_start(out=out_k[0][0:1, 0:4], in_=qt_sb[0:1, 0:4])
                return

            # ---- ck^T per chunk: 4 PE transposes + PSUM->SBUF copy
            # (ACT for even chunks, DVE for odd: both copy engines in parallel)
            ckt_sb = []
            for c in range(4):
                ct_ps = pck.tile([D, 4 * N], F32, name=f"ctp{c}")
                for j in range(4):
                    nc.tensor.transpose(out=ct_ps[:, j * N:(j + 1) * N],
                                        in_=ck_sb[c][:, j * D:(j + 1) * D],
                                        identity=ident[:])
                t = wp.tile([D, 4 * N], F32, name=f"ckt{c}")
                if c % 2 == 0:
                    nc.scalar.copy(out=t[:], in_=ct_ps[:])
                else:
                    nc.vector.tensor_copy(out=t[:], in_=ct_ps[:])
                ckt_sb.append(t)

            # ---- scoresT[n, (p h)]: 16 matvecs, one per head ----
            scoresT_ps = ps.tile([N, GH], F32, tag="s")
            for c in range(4):
                p, hh = c // 2, (c % 2) * 4
                for j in range(4):
                    i = p * HPG + hh + j
                    nc.tensor.matmul(out=scoresT_ps[:, i:i + 1],
                                     lhsT=ckt_sb[c][:, j * N:(j + 1) * N],
                                     rhs=qt_sb[:, i:i + 1],
                                     start=True, stop=True)

            if stage == 22:
                nc.sync.dma_start(out=out_k[0][0:1, 0:4],
                                  in_=ckt_sb[3][0:1, 0:4])
                return

            # ---- softmax over n without max-subtraction (scores ~ N(0,1)
            # after scaling, exp is overflow-safe) ----
            ecolT = wp.tile([N, GH], F32)
            nc.scalar.activation(out=ecolT[:], in_=scoresT_ps[:],
                                 func=Act.Exp, scale=SCALE)
            # z[h] = sum_n e^T[n, h] on PE (parallel with the e-transpose)
            z_ps = ps.tile([GH, 1], F32, tag="s")
            nc.tensor.matmul(out=z_ps[:], lhsT=ecolT[:], rhs=onescol[:],
                             start=True, stop=True)
            e_ps = ps.tile([GH, N], F32, tag="s")
            nc.tensor.transpose(out=e_ps[:], in_=ecolT[:], identity=ident[:])
            e_sb = wp.tile([GH, N], F32)
            nc.vector.tensor_copy(out=e_sb[:], in_=e_ps[:])
            rz = wp.tile([GH, 1], F32)
            nc.vector.reciprocal(out=rz[:, :1], in_=z_ps[:, :1])
            if stage == 23:
                nc.sync.dma_start(out=out_k[0][0:1, 0:4], in_=e_sb[0:1, 0:4])
                nc.scalar.dma_start(out=out_v[0][0:1, 0:1], in_=rz[0:1, 0:1])
                return

            # ---- pooled (x8, order-preserving) in both shapes per pair:
            # B_p[r, c] = A_p[c] (rows) and A_p[c] (column), identical
            # contraction order so the diagonal matches to rounding ----
            rzmask = wp.tile([GH, PAIRS], F32)
            nc.vector.tensor_scalar(out=rzmask[:], in0=pmask[:],
                                    scalar1=rz[:, :1], scalar2=None,
                                    op0=Alu.mult)
            b2_ps = pm.tile([N, PAIRS * N], F32, tag="mid")
            a2_ps = ps.tile([N, PAIRS], F32, tag="s")
            for p in range(PAIRS):
                nc.tensor.matmul(out=b2_ps[:, p * N:(p + 1) * N],
                                 lhsT=rzmask[:, p:p + 1].to_broadcast([GH, N]),
                                 rhs=e_sb[:], start=True, stop=True)
            nc.tensor.matmul(out=a2_ps[:], lhsT=e_sb[:], rhs=rzmask[:],
                             start=True, stop=True)
            a2_sb = wp.tile([N, PAIRS], F32)
            nc.vector.tensor_copy(out=a2_sb[:], in_=a2_ps[:])
            if stage == 24:
                nc.sync.dma_start(out=out_k[0][0:1, 0:2], in_=a2_sb[0:1, 0:2])
                return

            # ---- rank[r] = #{c: A[c] > A[r]} + #{c < r: A[c] == A[r]}
            # (diagonal excluded via noti; fused multiply+reduce on DVE) ----
            rank2 = wp.tile([128, PAIRS], F32)
            gj = wp.tile([128, PAIRS * N], F32)
            ej = wp.tile([128, PAIRS * N], F32)
            gt = wp.tile([128, PAIRS * N], F32)
            for p in range(PAIRS):
                sl = slice(p * N, (p + 1) * N)
                nc.vector.tensor_scalar(out=gj[:, sl], in0=b2_ps[:, sl],
                                        scalar1=a2_sb[:, p:p + 1], scalar2=None,
                                        op0=Alu.is_gt)
                nc.vector.tensor_scalar(out=ej[:, sl], in0=b2_ps[:, sl],
                                        scalar1=a2_sb[:, p:p + 1], scalar2=None,
                                        op0=Alu.is_equal)
            nc.vector.tensor_tensor(out=gj[:], in0=gj[:], in1=noti2[:],
                                    op=Alu.mult)
            nc.vector.tensor_tensor(out=ej[:], in0=ej[:], in1=tri2[:],
                                    op=Alu.mult)
            nc.vector.tensor_tensor(out=gt[:], in0=gj[:], in1=ej[:],
                                    op=Alu.add)
            for p in range(PAIRS):
                sl = slice(p * N, (p + 1) * N)
                nc.vector.tensor_reduce(out=rank2[:, p:p + 1], in_=gt[:, sl],
                                        op=Alu.add, axis=Ax.X)

            if stage == 25:
                nc.sync.dma_start(out=out_k[0][0:1, 0:2], in_=rank2[0:1, 0:2])
                return

            # ---- selection matrix -> chunk bases -> global chunk ids ----
            sel2 = wp.tile([128, PAIRS * NCHUNK], F32)
            chunk_ps = ps.tile([NCHUNK, PAIRS], F32, tag="s")
            for p in range(PAIRS):
                sl = slice(p * NCHUNK, (p + 1) * NCHUNK)
                nc.vector.tensor_scalar(out=sel2[:, sl], in0=iotabh[:],
                                        scalar1=rank2[:, p:p + 1], scalar2=None,
                                        op0=Alu.is_equal)
                nc.tensor.matmul(out=chunk_ps[:, p:p + 1], lhsT=sel2[:, sl],
                                 rhs=pvecr[:], start=True, stop=True)
            idxi = wp.tile([NCHUNK, PAIRS], I32)
            nc.vector.tensor_tensor(out=idxi[:], in0=chunk_ps[:], in1=cvec2[:],
                                    op=Alu.add)
            if dbg_i is not None:
                nc.sync.dma_start(out=dbg_i[:], in_=idxi[:])

            if stage == 2:
                nc.gpsimd.dma_start(out=out_k[0][0:1, 0:128],
                                    in_=idxi[:, 0:1].rearrange("a b -> b a"))
                return

            # ---- gathers (SWDGE q0), then stores (SWDGE q1); all gathers
            # are issued before any store so the gpsimd stream never stalls
            # a later gather behind a store's completion wait ----
            ksel, vsel = [], []
            for p in range(PAIRS):
                kt = wp.tile([128, NSEL * BS * D // 128], F32, name=f"ks{p}")
                nc.gpsimd.indirect_dma_start(
                    out=kt[:], out_offset=None, in_=k_flat,
                    in_offset=bass.IndirectOffsetOnAxis(ap=idxi[:, p:p + 1],
                                                        axis=0))
                ksel.append(kt)
                vt = wp.tile([128, NSEL * BS * D // 128], F32, name=f"vs{p}")
                nc.gpsimd.indirect_dma_start(
                    out=vt[:], out_offset=None, in_=v_flat,
                    in_offset=bass.IndirectOffsetOnAxis(ap=idxi[:, p:p + 1],
                                                        axis=0))
                vsel.append(vt)

            if stage == 3:
                nc.sync.dma_start(out=out_k[0][0:1, 0:4], in_=ksel[0][0:1, 0:4])
                nc.scalar.dma_start(out=out_v[0][0:1, 0:4], in_=vsel[0][0:1, 0:4])
                return

            for p in range(PAIRS):
                nc.sync.dma_start(
                    out=out_k[p].rearrange("(c r) d -> c (r d)", r=CHUNK // 4),
                    in_=ksel[p][:])
                nc.scalar.dma_start(
                    out=out_v[p].rearrange("(c r) d -> c (r d)", r=CHUNK // 4),
                    in_=vsel[p][:])


def _consts():
    return {}


def kernel(query, compressed_keys, keys, values):
    global LAST_RESULT
    from concourse.bass_utils import run_bass_kernel_spmd

    query = np.asarray(query, dtype=np.float32)
    compressed_keys = np.asarray(compressed_keys, dtype=np.float32)
    keys = np.asarray(keys, dtype=np.float32)
    values = np.asarray(values, dtype=np.float32)

    key = (os.environ.get("KREPEAT", "1"), os.environ.get("KEMPTY", "0"),
           os.environ.get("KSTAGE", "0"), os.environ.get("KDEBUG", "0"))
    if key not in _CACHE:
        _CACHE[key] = _build_nc()
    nc = _CACHE[key]

    in_maps = []
    for core in range(NCORES):
        bs, gs = [], []
        for j in range(PAIRS):
            f = PAIRS * core + j
            bs.append(f // G)
            gs.append(f % G)
        q_s = np.stack([query[b, g * HPG:(g + 1) * HPG, -1, :]
                        for b, g in zip(bs, gs)])
        ck_s = np.stack([compressed_keys[b, g * HPG:(g + 1) * HPG]
                         for b, g in zip(bs, gs)])
        k_s = np.stack([keys[b, g] for b, g in zip(bs, gs)])
        v_s = np.stack([values[b, g] for b, g in zip(bs, gs)])
        im = {"q_in": np.ascontiguousarray(q_s),
              "ck_in": np.ascontiguousarray(ck_s),
              "k_in": np.ascontiguousarray(k_s),
              "v_in": np.ascontiguousarray(v_s)}
        in_maps.append(im)

    res = run_bass_kernel_spmd(nc, in_maps, list(range(NCORES)))
    LAST_RESULT = res

    sel_k = np.empty((B, G, NSEL * BS, D), dtype=np.float32)
    sel_v = np.empty((B, G, NSEL * BS, D), dtype=np.float32)
    for core in range(NCORES):
        for j in range(PAIRS):
            f = PAIRS * core + j
            b, g = f // G, f % G
            sel_k[b, g] = res.results[core]["out_k"][j]
            sel_v[b, g] = res.results[core]["out_v"][j]
    return sel_k, sel_v


# revision 28
# speedup vs baseline: 1.3635x; 1.1455x over previous
"""Blockwise K/V selector (sparse attention) on 8 Trainium2 NeuronCores.

Full computation on device:
  scores = q . compressed_keys / sqrt(D)  -> softmax -> GQA mean-pool over
  heads -> top-16 blocks (rank trick, no sort) -> indirect-DMA gather of the
  selected 64-row K/V blocks.

Sharding: the 16 (b, g) pairs are fully independent; each of the 8 cores
processes 2 pairs (pure data parallel, no collectives).

Pipeline (latency-optimized; the gather/store tail overlaps the next
iteration's load/compute phase, so the critical path is loads + the
score->index chain):
  - ck loads split into 4 chunks alternating across the two HWDGE rings
    (SP + ACT) so per-chunk transposes overlap the remaining loads.
  - both (b,g) pairs share one merged instruction chain (scores [n,16],
    one exp, one e-transpose, fused rank reduction via tensor_tensor_reduce).
  - all mask/iota constants generated on-chip (no constant DMA).
  - gathers on SWDGE queue 0, K/V stores on SWDGE queue 1 (issued after all
    gathers in the gpsimd stream to avoid head-of-line blocking), keeping the
    HWDGE rings free for the next iteration's loads.
"""
import os
import numpy as np

B = 4
H = 32
G = 4
HPG = H // G          # 8 heads per query group
PAIRS = 2             # (b, g) pairs per core
N = 128               # number of compressed keys / key blocks
D = 128               # head dim
S = 8192              # kv sequence length
BS = 64               # block size
NSEL = 16             # selected blocks
NCORES = 8
# gather granularity: 8 rows = 4 KiB per index. The indirect-DMA DGE maps
# one index to one dest SBUF partition, so the per-index span must equal one
# partition line of the dest tile (4 KiB) — larger spans corrupt on HW.
CHUNK = 8
NCHUNK = NSEL * BS // CHUNK   # 128 chunks per pair
RPB = BS // CHUNK     # chunks per block (8)
SCALE = 1.0 / float(D) ** 0.5
GH = PAIRS * HPG      # 16 heads handled per core

_CACHE = {}
LAST_RESULT = None    # BassKernelResults of the most recent run (for test.py)


def _build_nc():
    import concourse.bass as bass
    import concourse.bacc as bacc
    import concourse.mybir as mybir
    import concourse.tile as tile

    F32 = mybir.dt.float32

    nc = bacc.Bacc("TRN2", target_bir_lowering=False, debug=False)

    q_in = nc.dram_tensor("q_in", [PAIRS, HPG, D], F32, kind="ExternalInput")
    ck_in = nc.dram_tensor("ck_in", [PAIRS, HPG, N, D], F32, kind="ExternalInput")
    k_in = nc.dram_tensor("k_in", [PAIRS, S, D], F32, kind="ExternalInput")
    v_in = nc.dram_tensor("v_in", [PAIRS, S, D], F32, kind="ExternalInput")
    out_k = nc.dram_tensor("out_k", [PAIRS, NSEL * BS, D], F32, kind="ExternalOutput")
    out_v = nc.dram_tensor("out_v", [PAIRS, NSEL * BS, D], F32, kind="ExternalOutput")
    dbg_i = None
    if int(os.environ.get("KDEBUG", "0")):
        dbg_i = nc.dram_tensor("dbg_i", [NCHUNK, PAIRS], mybir.dt.int32,
                               kind="ExternalOutput")

    # flat chunk views for the gathers: [2*1024 chunks, 1024 elems]
    k_flat = k_in[:].rearrange("b (c r) d -> (b c) (r d)", r=CHUNK)
    v_flat = v_in[:].rearrange("b (c r) d -> (b c) (r d)", r=CHUNK)

    # KREPEAT>1 builds the pipeline several times (serialized by the
    # TileContext exit barrier) so device time can be measured as the
    # marginal wall-clock per repeat. KEMPTY=1 emits no-op contexts for
    # calibrating the barrier cost.
    repeat = int(os.environ.get("KREPEAT", "1"))
    empty = bool(int(os.environ.get("KEMPTY", "0")))
    # KSTAGE: 0=full, 1=loads only, 2=+compute/idx, 3=+gathers (no stores)
    for _rep in range(repeat):
        _emit_once(nc, tc_mod=tile, bassmod=bass, mybirmod=mybir, empty=empty,
                   tensors=(q_in, ck_in, k_flat, v_flat, out_k, out_v, dbg_i))

    nc.compile()
    return nc


def _emit_once(nc, tc_mod, bassmod, mybirmod, empty, tensors):
    bass = bassmod
    mybir = mybirmod
    tile = tc_mod
    (q_in, ck_in, k_flat, v_flat, out_k, out_v, dbg_i) = tensors
    stage = int(os.environ.get("KSTAGE", "0"))
    from concourse.masks import make_identity
    F32 = mybir.dt.float32
    I32 = mybir.dt.int32
    Alu = mybir.AluOpType
    Act = mybir.ActivationFunctionType
    Ax = mybir.AxisListType

    with tile.TileContext(nc) as tc:
        if empty:
            with tc.tile_pool(name="noop", bufs=1) as np_:
                t = np_.tile([1, 1], F32)
                nc.vector.memset(t[:], 0.0)
            return
        with tc.tile_pool(name="consts", bufs=1) as cp, \
             tc.tile_pool(name="work", bufs=2) as wp, \
             tc.tile_pool(name="pckt", bufs=1, space="PSUM") as pck, \
             tc.tile_pool(name="pmid", bufs=2, space="PSUM") as pm, \
             tc.tile_pool(name="psml", bufs=2, space="PSUM") as ps:

            # ---- loads: q leads the SP ring; ck in 4 chunks alternating
            # across the SP / ACT HWDGE rings ----
            q_sb = wp.tile([GH, D], F32)
            nc.sync.dma_start(out=q_sb[:], in_=q_in[:].rearrange("b h d -> (b h) d"))
            ck_sb = []
            for c in range(4):
                p, hh = c // 2, (c % 2) * 4
                t = wp.tile([128, 4 * D], F32, name=f"ck{c}")
                eng = nc.sync if c % 2 == 0 else nc.scalar
                eng.dma_start(
                    out=t[:].rearrange("n (h d) -> n h d", h=4),
                    in_=ck_in[p, hh:hh + 4].rearrange("h n d -> n h d"))
                ck_sb.append(t)

            # ---- constants generated on-chip (cheap; overlaps the loads) ----
            ident = cp.tile([128, 128], F32)
            make_identity(nc, ident[:])
            # tri2[r, (p c)] = 1 iff c < r  (strict lower triangle, both pairs)
            tri2 = cp.tile([128, PAIRS * 128], F32)
            nc.gpsimd.memset(tri2[:], 1.0)
            nc.gpsimd.affine_select(out=tri2[:], in_=tri2[:],
                                    compare_op=Alu.is_gt, fill=0.0, base=0,
                                    pattern=[[0, PAIRS], [-1, 128]],
                                    channel_multiplier=1)
            # noti2[r, (p c)] = 1 iff c != r (diagonal exclusion, both pairs)
            noti2 = cp.tile([128, PAIRS * 128], F32)
            nc.gpsimd.memset(noti2[:], 1.0)
            nc.gpsimd.affine_select(out=noti2[:], in_=noti2[:],
                                    compare_op=Alu.not_equal,
                                    fill=0.0, base=0,
                                    pattern=[[0, PAIRS], [-1, 128]],
                                    channel_multiplier=1)
            # iotabh[r, c] = c // RPB
            iotabh = cp.tile([128, NCHUNK], F32)
            nc.gpsimd.iota(iotabh[:], pattern=[[1, NSEL], [0, RPB]], base=0,
                           channel_multiplier=0,
                           allow_small_or_imprecise_dtypes=True)
            # pvecr[r] = RPB * r
            pvecr = cp.tile([128, 1], F32)
            nc.gpsimd.iota(pvecr[:], pattern=[[0, 1]], base=0,
                           channel_multiplier=RPB,
                           allow_small_or_imprecise_dtypes=True)
            onescol = cp.tile([128, 1], F32)
            nc.gpsimd.memset(onescol[:], 1.0)
            # cvec2[c, p] = (c % RPB) + p * (S // CHUNK)
            it2 = cp.tile([128, PAIRS], I32)
            nc.gpsimd.iota(it2[:], pattern=[[0, PAIRS]], base=0,
                           channel_multiplier=1)
            m8 = cp.tile([128, PAIRS], I32)
            nc.vector.tensor_scalar(out=m8[:], in0=it2[:], scalar1=RPB - 1,
                                    scalar2=None, op0=Alu.bitwise_and)
            pb = cp.tile([128, PAIRS], I32)
            nc.gpsimd.iota(pb[:], pattern=[[S // CHUNK, PAIRS]], base=0,
                           channel_multiplier=0)
            cvec2 = cp.tile([128, PAIRS], F32)
            nc.vector.tensor_tensor(out=cvec2[:], in0=m8[:], in1=pb[:],
                                    op=Alu.add)
            # pmask[h, p] = 1 iff head h belongs to pair p (PE operands must
            # start at partition 0, so pooled matmuls contract over all 16
            # heads with the other pair's terms masked to exact zeros)
            # pmask[h, p] = 1 iff 0 <= h - 8p <= 7, via two affine selects
            pmask = cp.tile([GH, PAIRS], F32)
            nc.gpsimd.memset(pmask[:], 1.0)
            nc.gpsimd.affine_select(out=pmask[:], in_=pmask[:],
                                    compare_op=Alu.is_ge, fill=0.0, base=0,
                                    pattern=[[-HPG, PAIRS]],
                                    channel_multiplier=1)
            nc.gpsimd.affine_select(out=pmask[:], in_=pmask[:],
                                    compare_op=Alu.is_ge, fill=0.0,
                                    base=HPG - 1,
                                    pattern=[[HPG, PAIRS]],
                                    channel_multiplier=-1)

            if stage == 1:
                sink = wp.tile([128, 1], F32)
                nc.vector.tensor_copy(out=sink[:], in_=ck_sb[3][:, 0:1])
                nc.sync.dma_start(out=out_k[0][0:1, 0:1], in_=sink[0:1, 0:1])
                return

            # ---- q^T via PE ----
            qt_ps = ps.tile([D, GH], F32, tag="s")
            nc.tensor.transpose(out=qt_ps[:], in_=q_sb[:],
                                identity=ident[0:GH, 0:GH])
            qt_sb = wp.tile([D, GH], F32)
            nc.vector.tensor_copy(out=qt_sb[:], in_=qt_ps[:])
            if stage == 21:
                nc.sync.dma_start(out=out_k[0][0:1, 0:4], in_=qt_sb[0:1, 0:4])
                return

            # ---- ck^T per chunk: 4 PE transposes + PSUM->SBUF copy
            # (ACT for even chunks, DVE for odd: both copy engines in parallel)
            ckt_sb = []
            for c in range(4):
                ct_ps = pck.tile([D, 4 * N], F32, name=f"ctp{c}")
                for j in range(4):
                    nc.tensor.transpose(out=ct_ps[:, j * N:(j + 1) * N],
                                        in_=ck_sb[c][:, j * D:(j + 1) * D],
                                        identity=ident[:])
                t = wp.tile([D, 4 * N], F32, name=f"ckt{c}")
                if c % 2 == 0:
                    nc.scalar.copy(out=t[:], in_=ct_ps[:])
                else:
                    nc.vector.tensor_copy(out=t[:], in_=ct_ps[:])
                ckt_sb.append(t)

            # ---- scoresT[n, (p h)]: 16 matvecs, one per head ----
            scoresT_ps = ps.tile([N, GH], F32, tag="s")
            for c in range(4):
                p, hh = c // 2, (c % 2) * 4
                for j in range(4):
                    i = p * HPG + hh + j
                    nc.tensor.matmul(out=scoresT_ps[:, i:i + 1],
                                     lhsT=ckt_sb[c][:, j * N:(j + 1) * N],
                                     rhs=qt_sb[:, i:i + 1],
                                     start=True, stop=True)

            if stage == 22:
                nc.sync.dma_start(out=out_k[0][0:1, 0:4],
                                  in_=ckt_sb[3][0:1, 0:4])
                return

            # ---- softmax over n without max-subtraction (scores ~ N(0,1)
            # after scaling, exp is overflow-safe) ----
            ecolT = wp.tile([N, GH], F32)
            nc.scalar.activation(out=ecolT[:], in_=scoresT_ps[:],
                                 func=Act.Exp, scale=SCALE)
            # z[h] = sum_n e^T[n, h] on PE (parallel with the e-transpose)
            z_ps = ps.tile([GH, 1], F32, tag="s")
            nc.tensor.matmul(out=z_ps[:], lhsT=ecolT[:], rhs=onescol[:],
                             start=True, stop=True)
            e_ps = ps.tile([GH, N], F32, tag="s")
            nc.tensor.transpose(out=e_ps[:], in_=ecolT[:], identity=ident[:])
            e_sb = wp.tile([GH, N], F32)
            nc.vector.tensor_copy(out=e_sb[:], in_=e_ps[:])
            rz = wp.tile([GH, 1], F32)
            nc.vector.reciprocal(out=rz[:, :1], in_=z_ps[:, :1])
            if stage == 23:
                nc.sync.dma_start(out=out_k[0][0:1, 0:4], in_=e_sb[0:1, 0:4])
                nc.scalar.dma_start(out=out_v[0][0:1, 0:1], in_=rz[0:1, 0:1])
                return

            # ---- pooled (x8, order-preserving) in both shapes per pair:
            # B_p[r, c] = A_p[c] (rows) and A_p[c] (column), identical
            # contraction order so the diagonal matches to rounding ----
            rzmask = wp.tile([GH, PAIRS], F32)
            nc.vector.tensor_scalar(out=rzmask[:], in0=pmask[:],
                                    scalar1=rz[:, :1], scalar2=None,
                                    op0=Alu.mult)
            b2_ps = pm.tile([N, PAIRS * N], F32, tag="mid")
            a2_ps = ps.tile([N, PAIRS], F32, tag="s")
            for p in range(PAIRS):
                nc.tensor.matmul(out=b2_ps[:, p * N:(p + 1) * N],
                                 lhsT=rzmask[:, p:p + 1].to_broadcast([GH, N]),
                                 rhs=e_sb[:], start=True, stop=True)
            nc.tensor.matmul(out=a2_ps[:], lhsT=e_sb[:], rhs=rzmask[:],
                             start=True, stop=True)
            a2_sb = a2_ps
            if stage == 24:
                a2c = wp.tile([N, PAIRS], F32)
                nc.vector.tensor_copy(out=a2c[:], in_=a2_ps[:])
                nc.sync.dma_start(out=out_k[0][0:1, 0:2], in_=a2c[0:1, 0:2])
                return

            # ---- rank[r] = #{c: A[c] > A[r]} + #{c < r: A[c] == A[r]}
            # (diagonal excluded via noti; fused multiply+reduce on DVE) ----
            rank2 = wp.tile([128, PAIRS], F32)
            gj = wp.tile([128, PAIRS * N], F32)
            ej = wp.tile([128, PAIRS * N], F32)
            gt = wp.tile([128, PAIRS * N], F32)
            for p in range(PAIRS):
                sl = slice(p * N, (p + 1) * N)
                nc.vector.tensor_scalar(out=gj[:, sl], in0=b2_ps[:, sl],
                                        scalar1=a2_sb[:, p:p + 1], scalar2=None,
                                        op0=Alu.is_gt)
                nc.vector.tensor_scalar(out=ej[:, sl], in0=b2_ps[:, sl],
                                        scalar1=a2_sb[:, p:p + 1], scalar2=None,
                                        op0=Alu.is_equal)
            nc.vector.tensor_tensor(out=gj[:], in0=gj[:], in1=noti2[:],
                                    op=Alu.mult)
            nc.vector.tensor_tensor(out=ej[:], in0=ej[:], in1=tri2[:],
                                    op=Alu.mult)
            nc.vector.tensor_tensor(out=gt[:], in0=gj[:], in1=ej[:],
                                    op=Alu.add)
            for p in range(PAIRS):
                sl = slice(p * N, (p + 1) * N)
                nc.vector.tensor_reduce(out=rank2[:, p:p + 1], in_=gt[:, sl],
                                        op=Alu.add, axis=Ax.X)

            if stage == 25:
                nc.sync.dma_start(out=out_k[0][0:1, 0:2], in_=rank2[0:1, 0:2])
                return

            # ---- selection matrix -> chunk bases -> global chunk ids ----
            sel2 = wp.tile([128, PAIRS * NCHUNK], F32)
            chunk_ps = ps.tile([NCHUNK, PAIRS], F32, tag="s")
            for p in range(PAIRS):
                sl = slice(p * NCHUNK, (p + 1) * NCHUNK)
                nc.vector.tensor_scalar(out=sel2[:, sl], in0=iotabh[:],
                                        scalar1=rank2[:, p:p + 1], scalar2=None,
                                        op0=Alu.is_equal)
                nc.tensor.matmul(out=chunk_ps[:, p:p + 1], lhsT=sel2[:, sl],
                                 rhs=pvecr[:], start=True, stop=True)
            idxi = wp.tile([NCHUNK, PAIRS], I32)
            nc.vector.tensor_tensor(out=idxi[:], in0=chunk_ps[:], in1=cvec2[:],
                                    op=Alu.add)
            if dbg_i is not None:
                nc.sync.dma_start(out=dbg_i[:], in_=idxi[:])

            if stage == 2:
                nc.gpsimd.dma_start(out=out_k[0][0:1, 0:128],
                                    in_=idxi[:, 0:1].rearrange("a b -> b a"))
                return

            # ---- gathers (SWDGE q0), then stores (SWDGE q1); all gathers
            # are issued before any store so the gpsimd stream never stalls
            # a later gather behind a store's completion wait ----
            ksel, vsel = [], []
            for p in range(PAIRS):
                kt = wp.tile([128, NSEL * BS * D // 128], F32, name=f"ks{p}")
                nc.gpsimd.indirect_dma_start(
                    out=kt[:], out_offset=None, in_=k_flat,
                    in_offset=bass.IndirectOffsetOnAxis(ap=idxi[:, p:p + 1],
                                                        axis=0))
                ksel.append(kt)
                vt = wp.tile([128, NSEL * BS * D // 128], F32, name=f"vs{p}")
                nc.gpsimd.indirect_dma_start(
                    out=vt[:], out_offset=None, in_=v_flat,
                    in_offset=bass.IndirectOffsetOnAxis(ap=idxi[:, p:p + 1],
                                                        axis=0))
                vsel.append(vt)

            if stage == 3:
                nc.sync.dma_start(out=out_k[0][0:1, 0:4], in_=ksel[0][0:1, 0:4])
                nc.scalar.dma_start(out=out_v[0][0:1, 0:4], in_=vsel[0][0:1, 0:4])
                return

            # stores on the SWDGE queue, after all gathers in the gpsimd
            # stream: the HWDGE rings then carry only loads, so the next
            # repeat's load phase never queues behind a store's completion
            # wait on the SP/ACT sequencers
            for p in range(PAIRS):
                nc.gpsimd.dma_start(
                    out=out_k[p].rearrange("(c r) d -> c (r d)", r=CHUNK // 4),
                    in_=ksel[p][:])
                nc.gpsimd.dma_start(
                    out=out_v[p].rearrange("(c r) d -> c (r d)", r=CHUNK // 4),
                    in_=vsel[p][:])


def _consts():
    return {}


def kernel(query, compressed_keys, keys, values):
    global LAST_RESULT
    from concourse.bass_utils import run_bass_kernel_spmd

    query = np.asarray(query, dtype=np.float32)
    compressed_keys = np.asarray(compressed_keys, dtype=np.float32)
    keys = np.asarray(keys, dtype=np.float32)
    values = np.asarray(values, dtype=np.float32)

    key = (os.environ.get("KREPEAT", "1"), os.environ.get("KEMPTY", "0"),
           os.environ.get("KSTAGE", "0"), os.environ.get("KDEBUG", "0"))
    if key not in _CACHE:
        _CACHE[key] = _build_nc()
    nc = _CACHE[key]

    in_maps = []
    for core in range(NCORES):
        bs, gs = [], []
        for j in range(PAIRS):
            f = PAIRS * core + j
            bs.append(f // G)
            gs.append(f % G)
        q_s = np.stack([query[b, g * HPG:(g + 1) * HPG, -1, :]
                        for b, g in zip(bs, gs)])
        ck_s = np.stack([compressed_keys[b, g * HPG:(g + 1) * HPG]
                         for b, g in zip(bs, gs)])
        k_s = np.stack([keys[b, g] for b, g in zip(bs, gs)])
        v_s = np.stack([values[b, g] for b, g in zip(bs, gs)])
        im = {"q_in": np.ascontiguousarray(q_s),
              "ck_in": np.ascontiguousarray(ck_s),
              "k_in": np.ascontiguousarray(k_s),
              "v_in": np.ascontiguousarray(v_s)}
        in_maps.append(im)

    res = run_bass_kernel_spmd(nc, in_maps, list(range(NCORES)))
    LAST_RESULT = res

    sel_k = np.empty((B, G, NSEL * BS, D), dtype=np.float32)
    sel_v = np.empty((B, G, NSEL * BS, D), dtype=np.float32)
    for core in range(NCORES):
        for j in range(PAIRS):
            f = PAIRS * core + j
            b, g = f // G, f % G
            sel_k[b, g] = res.results[core]["out_k"][j]
            sel_v[b, g] = res.results[core]["out_v"][j]
    return sel_k, sel_v
